# revision 10
# baseline (speedup 1.0000x reference)
"""Causal self-attention on 8 NeuronCores (TRN2), tensor-parallel over heads.

Reference: y = proj(softmax(causal(Q K^T / sqrt(64))) V) with
B=4, T=2048, D=1024, H=16 heads, head_dim=64.

Sharding: each core owns 2 heads (a 128-column slice of the Q/K/V
projections and the matching 128 rows of w_proj) for all batches. Each
core emits a partial [B*T, D] output; the host sums the 8 partials
(row-parallel matmul unshard) and reshapes to [B, T, D].

All PE operands are bf16 (fp32 PSUM accumulation). V is produced
directly in token-major layout (x chunk stationary), so no PE
transposes are needed. The whole schedule is software-pipelined: the
attention kk-loop for batch b emits S of step i+1 before AV of step i,
and qkv(b+1) / out_proj(b-1) matmul groups are interleaved into the
stream as PE filler so the tensor engine never idles behind the
scalar-engine exp (idle PE triggers HAM re-throttle to half clock).
The softmax divide of each batch's last query chunk is carried into
the next batch's stream so its reciprocal never blocks the gpsimd
queue. Only Exp/Copy activations are used - a single table load.
"""

import sys

for _p in ("/opt/trn_rl_repo",):
    if _p not in sys.path:
        sys.path.insert(0, _p)

import ml_dtypes
import numpy as np

import concourse.bass as bass
import concourse.bacc as bacc
import concourse.mybir as mybir
from concourse import tile
from concourse.bass_utils import run_bass_kernel_spmd

B, T, D, H = 4, 2048, 1024, 16
HD = D // H           # 64 head dim
NCORES = 8
HPC = H // NCORES     # 2 heads per core
CW = HPC * HD         # 128: per-core qkv column slice width
BT = B * T            # 8192 tokens
KC = D // 128         # 8 contraction chunks for the qkv projection
NQ = 512              # query chunk
NG = NQ // 128        # 4 key-tiles per S^T group
F32 = mybir.dt.float32
BF16 = mybir.dt.bfloat16
EXP = mybir.ActivationFunctionType.Exp
BF16NP = np.dtype(ml_dtypes.bfloat16)

VST = HPC * (HD + 1)  # 130: V tile stride (per head: 64 cols + ones col)
NSTEP = sum(NG * (jq + 1) for jq in range(T // NQ))  # 40 attention steps/batch


def build_kernel():
    nc = bacc.Bacc("TRN2", target_bir_lowering=False, debug=False)

    xT = nc.dram_tensor("xT", [D, BT], BF16, kind="ExternalInput")
    # wqkv packed on host as [128, KC, 3*CW]: (kc,:) = rows kc*128..+128 of
    # [w_q_slice | w_k_slice | w_v_slice]
    wqkv = nc.dram_tensor("wqkv", [128, KC * 3 * CW], BF16, kind="ExternalInput")
    wp = nc.dram_tensor("wp", [CW, D], BF16, kind="ExternalInput")
    out = nc.dram_tensor("out", [BT, D], BF16, kind="ExternalOutput")

    with tile.TileContext(nc) as tc:
        _body(tc, xT.ap(), wqkv.ap(), wp.ap(), out.ap())
    nc.compile()
    return nc


def _body(tc, xT, wqkv, wp, out):
    nc = tc.nc
    with (
        tc.tile_pool(name="const", bufs=1) as const,
        tc.tile_pool(name="xin", bufs=2) as xin,
        tc.tile_pool(name="qk", bufs=2) as qkpool,
        tc.tile_pool(name="vb", bufs=2) as vbpool,
        tc.tile_pool(name="pt", bufs=3) as ptpool,
        tc.tile_pool(name="yt", bufs=2) as ytpool,
        tc.tile_pool(name="dn", bufs=2) as dnpool,
        tc.tile_pool(name="os", bufs=2) as ospool,
        tc.tile_pool(name="pst", bufs=2, space="PSUM") as pst,
        tc.tile_pool(name="pav", bufs=1, space="PSUM") as pav,
        tc.tile_pool(name="psm", bufs=2, space="PSUM") as psm,
    ):
        # ---- constants ----
        wq_sb = const.tile([128, KC, 3 * CW], BF16, tag="wqkv")
        nc.sync.dma_start(wq_sb[:], wqkv.rearrange("p (k c) -> p k c", k=KC))
        wp_sb = const.tile([CW, D], BF16, tag="wp")
        nc.sync.dma_start(wp_sb[:], wp[:])
        ones32 = const.tile([128, (T // 128) * HPC], BF16, tag="ones32")
        nc.gpsimd.memset(ones32[:], 1.0)
        # causal mask for the 128-wide diagonal blocks: mask[p, j] = j >= p,
        # stored twice so one DVE multiply covers both heads' strided blocks
        mask2 = const.tile([128, HPC * 128], BF16, tag="mask2")
        nc.gpsimd.memset(mask2[:], 1.0)
        nc.gpsimd.affine_select(
            out=mask2[:],
            in_=mask2[:],
            pattern=[[0, HPC], [1, 128]],
            channel_multiplier=-1,
            base=0,
            compare_op=mybir.AluOpType.is_ge,
            fill=0.0,
        )
        scale = 1.0 / float(np.sqrt(HD))

        xts, qkvs, yts = {}, {}, {}

        def load_chunks(b):
            def go():
                xt = xin.tile([128, KC, T], BF16, tag="xt")
                xts[b] = xt
                tok0 = b * T
                for kc in range(KC):
                    nc.sync.dma_start(
                        xt[:, kc, :],
                        xT[kc * 128 : (kc + 1) * 128, tok0 : tok0 + T],
                    )
            return [go]

        def qkv_chunks(b):
            """Closures, each emitting one PSUM-group of qkv(b) work."""
            def alloc():
                qt = qkpool.tile([128, T], BF16, tag="qt")
                kt = qkpool.tile([128, T], BF16, tag="kt")
                vb = vbpool.tile([128, (T // 128) * VST], BF16, tag="vb")
                qkvs[b] = (qt, kt, vb)

            chunks = [alloc]
            for ch in range(T // NQ):
                # Q^T / K^T m-tiles (weight-stationary, token-moving)
                for m in (0, 1):
                    def qk_go(ch=ch, m=m):
                        qt, kt, vb = qkvs[b]
                        xt = xts[b]
                        sl = slice(ch * NQ, (ch + 1) * NQ)
                        ps = psm.tile([128, NQ], F32, tag="ps")
                        for kc in range(KC):
                            nc.tensor.matmul(
                                ps[:],
                                wq_sb[:, kc, m * CW : (m + 1) * CW],
                                xt[:, kc, sl],
                                start=(kc == 0),
                                stop=(kc == KC - 1),
                            )
                        nc.vector.tensor_copy((qt, kt)[m][:, sl], ps[:])
                    chunks.append(qk_go)
                # V in token-major layout directly: x 128-token blocks
                # stationary, w_v moving; 4 blocks accumulate into one psum
                # tile, one strided scatter into vb per 512-token chunk
                def v_go(ch=ch):
                    qt, kt, vb = qkvs[b]
                    xt = xts[b]
                    pv = psm.tile([128, NQ], F32, tag="ps")
                    for blk in range(NG):
                        t0 = ch * NQ + blk * 128
                        for kc in range(KC):
                            nc.tensor.matmul(
                                pv[:, blk * CW : (blk + 1) * CW],
                                xt[:, kc, t0 : t0 + 128],
                                wq_sb[:, kc, 2 * CW : 3 * CW],
                                start=(kc == 0),
                                stop=(kc == KC - 1),
                            )
                    dstv = bass.AP(
                        vb.tensor,
                        vb[:].offset + ch * NG * VST,
                        [vb[:].ap[0], [VST, NG], [HD + 1, HPC], [1, HD]],
                    )
                    srcv = pv[:].rearrange("p (t h d) -> p t h d", t=NG, h=HPC)
                    nc.vector.tensor_copy(dstv, srcv)
                chunks.append(v_go)

            def ones_go():
                qt, kt, vb = qkvs[b]
                onesv = bass.AP(
                    vb.tensor,
                    vb[:].offset + HD,
                    [vb[:].ap[0], [HD + 1, (T // 128) * HPC]],
                )
                nc.vector.tensor_copy(onesv, ones32[:])
            chunks.append(ones_go)
            return chunks

        def finalize_pieces(yt, jq, ytu):
            # divide O^T rows by the denominator row (broadcast to 64 parts).
            # Emitted as closures dripped one-per-step into the stream so the
            # reciprocal does not convoy PSUM evacuations on the DVE queue.
            q0 = jq * NQ
            dn = dnpool.tile([1, HPC * NQ], F32, tag="dn")
            dnb = dnpool.tile([HD, HPC * NQ], F32, tag="dnb")
            pieces = [
                lambda: nc.vector.reciprocal(
                    dn[:, 0:NQ], ytu[HD : HD + 1, 0:NQ]
                ),
                lambda: nc.vector.reciprocal(
                    dn[:, NQ : 2 * NQ], ytu[HD : HD + 1, NQ : 2 * NQ]
                ),
                lambda: nc.gpsimd.partition_broadcast(dnb[:], dn[:]),
            ]
            for h in range(HPC):
                pieces.append(lambda h=h: nc.vector.tensor_mul(
                    yt[h * HD : (h + 1) * HD, q0 : q0 + NQ],
                    ytu[0:HD, h * NQ : (h + 1) * NQ],
                    dnb[:, h * NQ : (h + 1) * NQ],
                ))
            return pieces

        def outproj_chunks(b):
            chunks = []
            for tt in range(T // 128):
                def go(tt=tt):
                    yt = yts[b]
                    tok0 = b * T
                    os_ = ospool.tile([128, D], BF16, tag="os")
                    for nn in range(D // NQ):
                        pp = psm.tile([128, NQ], F32, tag="ps")
                        nc.tensor.matmul(
                            pp[:],
                            yt[:, tt * 128 : (tt + 1) * 128],
                            wp_sb[:, nn * NQ : (nn + 1) * NQ],
                            start=True,
                            stop=True,
                        )
                        if nn == 0:
                            nc.vector.tensor_copy(
                                os_[:, nn * NQ : (nn + 1) * NQ], pp[:]
                            )
                        else:
                            nc.scalar.copy(
                                os_[:, nn * NQ : (nn + 1) * NQ], pp[:]
                            )
                    nc.sync.dma_start(
                        out[tok0 + tt * 128 : tok0 + (tt + 1) * 128, :], os_[:]
                    )
                chunks.append(go)
            return chunks

        def attention_stream(b, yt, state):
            """Generator: yields after each kk step so the driver can
            interleave filler (qkv of b+1 / out-proj of b-1) into the PE
            stream. h0 lives in SBUF partitions 0-63, h1 in 64-127; the
            S-exp of step i+1 is emitted before the AV of step i."""
            qt, kt, vb = qkvs[b]

            def emit_s(jq, kk):
                q0 = jq * NQ
                i = kk - NG * jq        # >= 0 on the diagonal run
                c0 = max(i, 0) * 128    # first valid q col in this chunk
                w = NQ - c0
                st = pst.tile([128, HPC * NQ], F32, tag="st")
                for h in range(HPC):
                    nc.tensor.matmul(
                        st[:, h * NQ + c0 : (h + 1) * NQ],
                        kt[h * HD : (h + 1) * HD, kk * 128 : (kk + 1) * 128],
                        qt[h * HD : (h + 1) * HD, q0 + c0 : q0 + NQ],
                        start=True,
                        stop=True,
                    )
                ptk = ptpool.tile([128, HPC * NQ], BF16, tag="pt")
                stv = bass.AP(st.tensor, st[:].offset + c0,
                              [st[:].ap[0], [NQ, HPC], [1, w]])
                ptv = bass.AP(ptk.tensor, ptk[:].offset + c0,
                              [ptk[:].ap[0], [NQ, HPC], [1, w]])
                nc.scalar.activation(ptv, stv, EXP, scale=scale)
                if i >= 0:
                    # zero q < kpart inside the 128-wide diagonal block via a
                    # DVE mask multiply - the gpsimd DSP is far too slow at
                    # semaphore waits to sit on the S->exp->mask->AV path
                    tri = bass.AP(ptk.tensor, ptk[:].offset + c0,
                                  [ptk[:].ap[0], [NQ, HPC], [1, 128]])
                    nc.vector.tensor_mul(
                        tri, tri,
                        mask2[:].rearrange("p (h j) -> p h j", h=HPC),
                    )
                return jq, kk, ptk, c0

            def emit_av(jq, kk, ptk, c0):
                nkk = NG * (jq + 1)
                if kk == 0:
                    av0 = pav.tile([128, NQ], F32, tag="av0")
                    av1 = pav.tile([128, NQ], F32, tag="av1")
                    state["avs"] = (av0, av1)
                avs = state["avs"]
                for h in range(HPC):
                    nc.tensor.matmul(
                        avs[h][0 : HD + 1, c0:NQ],
                        vb[:, kk * VST + h * (HD + 1) :
                             kk * VST + (h + 1) * (HD + 1)],
                        ptk[:, h * NQ + c0 : (h + 1) * NQ],
                        start=(kk == 0),
                        stop=(kk == nkk - 1),
                    )
                if kk == nkk - 1:
                    # evacuate PSUM promptly (frees the av slots); rows 0..63
                    # are the unnormalized O^T, row 64 the denominator
                    ytu = dnpool.tile([HD + 1, HPC * NQ], F32, tag="ytu")
                    nc.vector.tensor_copy(
                        ytu[0 : HD + 1, 0:NQ], avs[0][0 : HD + 1, :]
                    )
                    nc.scalar.copy(
                        ytu[0 : HD + 1, NQ : 2 * NQ], avs[1][0 : HD + 1, :]
                    )
                    if jq == T // NQ - 1:
                        state["pending"] = (jq, ytu)
                    else:
                        state["npieces"].extend(finalize_pieces(yt, jq, ytu))

            prev = None
            idx = 0
            for jq in range(T // NQ):
                for kk in range(NG * (jq + 1)):
                    cur = emit_s(jq, kk)
                    if prev is not None:
                        emit_av(*prev)
                    prev = cur
                    yield
            emit_av(*prev)

        # ---- prologue: x(0), x(1) loads and qkv(0) emitted straight ----
        for go in load_chunks(0) + load_chunks(1) + qkv_chunks(0):
            go()

        carry = None
        for b in range(B):
            yt = ytpool.tile([128, T], BF16, tag="yt")
            yts[b] = yt
            fillers = []
            if b + 2 < B:
                fillers += load_chunks(b + 2)
            if b + 1 < B:
                fillers += qkv_chunks(b + 1)
            if b - 1 >= 0:
                fillers += outproj_chunks(b - 1)
            state = {"pending": None, "avs": None, "npieces": []}
            fi, step = 0, 0
            for _ in attention_stream(b, yt, state):
                step += 1
                if step == NG and carry is not None:
                    # previous batch's last softmax divide enters the drip
                    # queue after this batch's first diagonal masks
                    state["npieces"].extend(finalize_pieces(*carry))
                    carry = None
                if state["npieces"]:
                    state["npieces"].pop(0)()
                # proportional pacing keeps late-dependency fillers late
                while fi * NSTEP < len(fillers) * step and fi < len(fillers):
                    fillers[fi]()
                    fi += 1
            for go in state["npieces"]:
                go()
            while fi < len(fillers):
                fillers[fi]()
                fi += 1
            jq3, ytu3 = state["pending"]
            carry = (yt, jq3, ytu3)

        # ---- epilogue ----
        for go in finalize_pieces(*carry):
            go()
        for go in outproj_chunks(B - 1):
            go()


_NC_CACHE = None


def make_in_maps(x, w_attn, w_proj):
    xT = np.ascontiguousarray(
        np.asarray(x, np.float32).reshape(BT, D).T
    ).astype(BF16NP)  # [D, BT]
    in_maps = []
    for c in range(NCORES):
        c0 = c * CW
        wq = w_attn[:, c0 : c0 + CW]
        wk = w_attn[:, D + c0 : D + c0 + CW]
        wv = w_attn[:, 2 * D + c0 : 2 * D + c0 + CW]
        wslice = np.concatenate([wq, wk, wv], axis=1)          # [D, 3*CW]
        wpacked = np.ascontiguousarray(
            np.asarray(wslice, np.float32)
            .reshape(KC, 128, 3 * CW)
            .transpose(1, 0, 2)
        ).reshape(128, KC * 3 * CW).astype(BF16NP)
        wpc = np.ascontiguousarray(
            np.asarray(w_proj[c0 : c0 + CW, :], np.float32)
        ).astype(BF16NP)                                        # [CW, D]
        in_maps.append({"xT": xT, "wqkv": wpacked, "wp": wpc})
    return in_maps


def kernel(x: np.ndarray, w_attn: np.ndarray, w_proj: np.ndarray) -> np.ndarray:
    global _NC_CACHE
    if _NC_CACHE is None:
        _NC_CACHE = build_kernel()
    nc = _NC_CACHE

    in_maps = make_in_maps(x, w_attn, w_proj)
    res = run_bass_kernel_spmd(nc, in_maps, core_ids=list(range(NCORES)))
    acc = np.zeros((BT, D), dtype=np.float32)
    for r in res.results:
        acc += np.asarray(r["out"], dtype=np.float32)
    return acc.reshape(B, T, D)


if __name__ == "__main__":
    inputs = {
        "x": np.random.randn(B, T, D).astype(np.float32),
        "w_attn": (np.random.randn(D, 3 * D) / np.sqrt(D)).astype(np.float32),
        "w_proj": (np.random.randn(D, D) / np.sqrt(D)).astype(np.float32),
    }
    y = kernel(**inputs)
    print(y.shape, y.dtype)


# revision 11
# speedup vs baseline: 1.2380x; 1.2380x over previous
"""Causal self-attention on 8 NeuronCores (TRN2), tensor-parallel over heads.

Reference: y = proj(softmax(causal(Q K^T / sqrt(64))) V) with
B=4, T=2048, D=1024, H=16 heads, head_dim=64.

Sharding: each core owns 2 heads (a 128-column slice of the Q/K/V
projections and the matching 128 rows of w_proj) for all batches. Each
core emits a partial [B*T, D] output; the host sums the 8 partials
(row-parallel matmul unshard) and reshapes to [B, T, D].

All PE operands are bf16 (fp32 PSUM accumulation). V is produced
directly in token-major layout (x chunk stationary), so no PE
transposes are needed. The whole schedule is software-pipelined: the
attention kk-loop for batch b emits S of step i+1 before AV of step i,
and qkv(b+1) / out_proj(b-1) matmul groups are interleaved into the
stream as PE filler so the tensor engine never idles behind the
scalar-engine exp (idle PE triggers HAM re-throttle to half clock).
The softmax divide of each batch's last query chunk is carried into
the next batch's stream so its reciprocal never blocks the gpsimd
queue. Only Exp/Copy activations are used - a single table load.
"""

import sys

for _p in ("/opt/trn_rl_repo",):
    if _p not in sys.path:
        sys.path.insert(0, _p)

import ml_dtypes
import numpy as np

import concourse.bass as bass
import concourse.bacc as bacc
import concourse.mybir as mybir
from concourse import tile
from concourse.bass_utils import run_bass_kernel_spmd

B, T, D, H = 4, 2048, 1024, 16
HD = D // H           # 64 head dim
NCORES = 8
HPC = H // NCORES     # 2 heads per core
CW = HPC * HD         # 128: per-core qkv column slice width
BT = B * T            # 8192 tokens
KC = D // 128         # 8 contraction chunks for the qkv projection
NQ = 512              # query chunk
NG = NQ // 128        # 4 key-tiles per S^T group
F32 = mybir.dt.float32
BF16 = mybir.dt.bfloat16
EXP = mybir.ActivationFunctionType.Exp
BF16NP = np.dtype(ml_dtypes.bfloat16)

VST = HPC * (HD + 1)  # 130: V tile stride (per head: 64 cols + ones col)
NSTEP = sum(NG * (jq + 1) for jq in range(T // NQ))  # 40 attention steps/batch


def build_kernel():
    nc = bacc.Bacc("TRN2", target_bir_lowering=False, debug=False)

    xT = nc.dram_tensor("xT", [D, BT], BF16, kind="ExternalInput")
    # wqkv packed on host as [128, KC, 3*CW]: (kc,:) = rows kc*128..+128 of
    # [w_q_slice | w_k_slice | w_v_slice]
    wqkv = nc.dram_tensor("wqkv", [128, KC * 3 * CW], BF16, kind="ExternalInput")
    wp = nc.dram_tensor("wp", [CW, D], BF16, kind="ExternalInput")
    out = nc.dram_tensor("out", [BT, D], BF16, kind="ExternalOutput")

    with tile.TileContext(nc) as tc:
        _body(tc, xT.ap(), wqkv.ap(), wp.ap(), out.ap())
    nc.compile()
    return nc


def _body(tc, xT, wqkv, wp, out):
    nc = tc.nc
    with (
        tc.tile_pool(name="const", bufs=1) as const,
        tc.tile_pool(name="xin", bufs=2) as xin,
        tc.tile_pool(name="qk", bufs=2) as qkpool,
        tc.tile_pool(name="vb", bufs=2) as vbpool,
        tc.tile_pool(name="pt", bufs=3) as ptpool,
        tc.tile_pool(name="yt", bufs=2) as ytpool,
        tc.tile_pool(name="dn", bufs=2) as dnpool,
        tc.tile_pool(name="os", bufs=2) as ospool,
        tc.tile_pool(name="pst", bufs=2, space="PSUM") as pst,
        tc.tile_pool(name="pav", bufs=1, space="PSUM") as pav,
        tc.tile_pool(name="psm", bufs=2, space="PSUM") as psm,
    ):
        # ---- constants ----
        wq_sb = const.tile([128, KC, 3 * CW], BF16, tag="wqkv")
        nc.sync.dma_start(wq_sb[:], wqkv.rearrange("p (k c) -> p k c", k=KC))
        wp_sb = const.tile([CW, D], BF16, tag="wp")
        nc.sync.dma_start(wp_sb[:], wp[:])
        ones32 = const.tile([128, (T // 128) * HPC], BF16, tag="ones32")
        nc.gpsimd.memset(ones32[:], 1.0)
        # causal mask for the 128-wide diagonal blocks: mask[p, j] = j >= p,
        # stored twice so one DVE multiply covers both heads' strided blocks
        mask2 = const.tile([128, HPC * 128], BF16, tag="mask2")
        nc.gpsimd.memset(mask2[:], 1.0)
        nc.gpsimd.affine_select(
            out=mask2[:],
            in_=mask2[:],
            pattern=[[0, HPC], [1, 128]],
            channel_multiplier=-1,
            base=0,
            compare_op=mybir.AluOpType.is_ge,
            fill=0.0,
        )
        scale = 1.0 / float(np.sqrt(HD))

        xts, qkvs, yts = {}, {}, {}

        def load_chunks(b):
            def go():
                xt = xin.tile([128, KC, T], BF16, tag="xt")
                xts[b] = xt
                tok0 = b * T
                for kc in range(KC):
                    nc.sync.dma_start(
                        xt[:, kc, :],
                        xT[kc * 128 : (kc + 1) * 128, tok0 : tok0 + T],
                    )
            return [go]

        def qkv_chunks(b):
            """Closures, each emitting one PSUM-group of qkv(b) work."""
            def alloc():
                qt = qkpool.tile([128, T], BF16, tag="qt")
                kt = qkpool.tile([128, T], BF16, tag="kt")
                vb = vbpool.tile([128, (T // 128) * VST], BF16, tag="vb")
                qkvs[b] = (qt, kt, vb)

            chunks = [alloc]
            for ch in range(T // NQ):
                # Q^T / K^T m-tiles (weight-stationary, token-moving)
                for m in (0, 1):
                    def qk_go(ch=ch, m=m):
                        qt, kt, vb = qkvs[b]
                        xt = xts[b]
                        sl = slice(ch * NQ, (ch + 1) * NQ)
                        ps = psm.tile([128, NQ], F32, tag="ps")
                        for kc in range(KC):
                            nc.tensor.matmul(
                                ps[:],
                                wq_sb[:, kc, m * CW : (m + 1) * CW],
                                xt[:, kc, sl],
                                start=(kc == 0),
                                stop=(kc == KC - 1),
                            )
                        nc.vector.tensor_copy((qt, kt)[m][:, sl], ps[:])
                    chunks.append(qk_go)
                # V in token-major layout directly: x 128-token blocks
                # stationary, w_v moving; 4 blocks accumulate into one psum
                # tile, one strided scatter into vb per 512-token chunk
                def v_go(ch=ch):
                    qt, kt, vb = qkvs[b]
                    xt = xts[b]
                    pv = psm.tile([128, NQ], F32, tag="ps")
                    for blk in range(NG):
                        t0 = ch * NQ + blk * 128
                        for kc in range(KC):
                            nc.tensor.matmul(
                                pv[:, blk * CW : (blk + 1) * CW],
                                xt[:, kc, t0 : t0 + 128],
                                wq_sb[:, kc, 2 * CW : 3 * CW],
                                start=(kc == 0),
                                stop=(kc == KC - 1),
                            )
                    dstv = bass.AP(
                        vb.tensor,
                        vb[:].offset + ch * NG * VST,
                        [vb[:].ap[0], [VST, NG], [HD + 1, HPC], [1, HD]],
                    )
                    srcv = pv[:].rearrange("p (t h d) -> p t h d", t=NG, h=HPC)
                    nc.vector.tensor_copy(dstv, srcv)
                chunks.append(v_go)

            def ones_go():
                qt, kt, vb = qkvs[b]
                onesv = bass.AP(
                    vb.tensor,
                    vb[:].offset + HD,
                    [vb[:].ap[0], [HD + 1, (T // 128) * HPC]],
                )
                nc.vector.tensor_copy(onesv, ones32[:])
            chunks.append(ones_go)
            return chunks

        def finalize_pieces(yt, jq, ytu):
            # divide O^T rows by the denominator row (broadcast to 64 parts).
            # Emitted as closures dripped one-per-step into the stream so the
            # reciprocal does not convoy PSUM evacuations on the DVE queue.
            q0 = jq * NQ
            dn = dnpool.tile([1, HPC * NQ], F32, tag="dn")
            dnb = dnpool.tile([HD, HPC * NQ], F32, tag="dnb")
            pieces = [
                lambda: nc.vector.reciprocal(dn[:], ytu[HD : HD + 1, :]),
                lambda: nc.gpsimd.partition_broadcast(dnb[:], dn[:]),
            ]
            for h in range(HPC):
                pieces.append(lambda h=h: nc.gpsimd.tensor_mul(
                    yt[h * HD : (h + 1) * HD, q0 : q0 + NQ],
                    ytu[0:HD, h * NQ : (h + 1) * NQ],
                    dnb[:, h * NQ : (h + 1) * NQ],
                ))
            return pieces

        def outproj_chunks(b):
            chunks = []
            for tt in range(T // 128):
                def go(tt=tt):
                    yt = yts[b]
                    tok0 = b * T
                    os_ = ospool.tile([128, D], BF16, tag="os")
                    for nn in range(D // NQ):
                        pp = psm.tile([128, NQ], F32, tag="ps")
                        nc.tensor.matmul(
                            pp[:],
                            yt[:, tt * 128 : (tt + 1) * 128],
                            wp_sb[:, nn * NQ : (nn + 1) * NQ],
                            start=True,
                            stop=True,
                        )
                        nc.scalar.copy(os_[:, nn * NQ : (nn + 1) * NQ], pp[:])
                    nc.sync.dma_start(
                        out[tok0 + tt * 128 : tok0 + (tt + 1) * 128, :], os_[:]
                    )
                chunks.append(go)
            return chunks

        def attention_stream(b, yt, state):
            """Generator: yields after each kk step so the driver can
            interleave filler (qkv of b+1 / out-proj of b-1) into the PE
            stream. h0 lives in SBUF partitions 0-63, h1 in 64-127; the
            S-exp of step i+1 is emitted before the AV of step i."""
            qt, kt, vb = qkvs[b]

            def emit_s(jq, kk):
                q0 = jq * NQ
                i = kk - NG * jq        # >= 0 on the diagonal run
                c0 = max(i, 0) * 128    # first valid q col in this chunk
                w = NQ - c0
                st = pst.tile([128, HPC * NQ], F32, tag="st")
                for h in range(HPC):
                    nc.tensor.matmul(
                        st[:, h * NQ + c0 : (h + 1) * NQ],
                        kt[h * HD : (h + 1) * HD, kk * 128 : (kk + 1) * 128],
                        qt[h * HD : (h + 1) * HD, q0 + c0 : q0 + NQ],
                        start=True,
                        stop=True,
                    )
                ptk = ptpool.tile([128, HPC * NQ], BF16, tag="pt")
                stv = bass.AP(st.tensor, st[:].offset + c0,
                              [st[:].ap[0], [NQ, HPC], [1, w]])
                ptv = bass.AP(ptk.tensor, ptk[:].offset + c0,
                              [ptk[:].ap[0], [NQ, HPC], [1, w]])
                nc.scalar.activation(ptv, stv, EXP, scale=scale)
                if i >= 0:
                    # zero q < kpart inside the 128-wide diagonal block via a
                    # DVE mask multiply - the gpsimd DSP is far too slow at
                    # semaphore waits to sit on the S->exp->mask->AV path
                    tri = bass.AP(ptk.tensor, ptk[:].offset + c0,
                                  [ptk[:].ap[0], [NQ, HPC], [1, 128]])
                    nc.vector.tensor_mul(
                        tri, tri,
                        mask2[:].rearrange("p (h j) -> p h j", h=HPC),
                    )
                return jq, kk, ptk, c0

            def emit_av(jq, kk, ptk, c0):
                nkk = NG * (jq + 1)
                if kk == 0:
                    av0 = pav.tile([128, NQ], F32, tag="av0")
                    av1 = pav.tile([128, NQ], F32, tag="av1")
                    state["avs"] = (av0, av1)
                avs = state["avs"]
                for h in range(HPC):
                    nc.tensor.matmul(
                        avs[h][0 : HD + 1, c0:NQ],
                        vb[:, kk * VST + h * (HD + 1) :
                             kk * VST + (h + 1) * (HD + 1)],
                        ptk[:, h * NQ + c0 : (h + 1) * NQ],
                        start=(kk == 0),
                        stop=(kk == nkk - 1),
                    )
                if kk == nkk - 1:
                    # evacuate PSUM promptly (frees the av slots); rows 0..63
                    # are the unnormalized O^T, row 64 the denominator
                    ytu = dnpool.tile([HD + 1, HPC * NQ], F32, tag="ytu")
                    nc.vector.tensor_copy(
                        ytu[0 : HD + 1, 0:NQ], avs[0][0 : HD + 1, :]
                    )
                    nc.scalar.copy(
                        ytu[0 : HD + 1, NQ : 2 * NQ], avs[1][0 : HD + 1, :]
                    )
                    if jq == T // NQ - 1:
                        state["pending"] = (jq, ytu)
                    else:
                        state["npieces"].extend(finalize_pieces(yt, jq, ytu))

            prev = None
            idx = 0
            for jq in range(T // NQ):
                for kk in range(NG * (jq + 1)):
                    cur = emit_s(jq, kk)
                    if prev is not None:
                        emit_av(*prev)
                    prev = cur
                    yield
            emit_av(*prev)

        # ---- prologue: x(0), x(1) loads and qkv(0) emitted straight ----
        for go in load_chunks(0) + load_chunks(1) + qkv_chunks(0):
            go()

        carry = None
        for b in range(B):
            yt = ytpool.tile([128, T], BF16, tag="yt")
            yts[b] = yt
            fillers = []
            if b + 2 < B:
                fillers += load_chunks(b + 2)
            if b + 1 < B:
                fillers += qkv_chunks(b + 1)
            if b - 1 >= 0:
                fillers += outproj_chunks(b - 1)
            state = {"pending": None, "avs": None, "npieces": []}
            fi, step = 0, 0
            for _ in attention_stream(b, yt, state):
                step += 1
                if step == NG and carry is not None:
                    # previous batch's last softmax divide enters the drip
                    # queue after this batch's first diagonal masks
                    state["npieces"].extend(finalize_pieces(*carry))
                    carry = None
                if state["npieces"]:
                    state["npieces"].pop(0)()
                # proportional pacing keeps late-dependency fillers late
                while fi * NSTEP < len(fillers) * step and fi < len(fillers):
                    fillers[fi]()
                    fi += 1
            for go in state["npieces"]:
                go()
            while fi < len(fillers):
                fillers[fi]()
                fi += 1
            jq3, ytu3 = state["pending"]
            carry = (yt, jq3, ytu3)

        # ---- epilogue ----
        for go in finalize_pieces(*carry):
            go()
        for go in outproj_chunks(B - 1):
            go()


_NC_CACHE = None


def make_in_maps(x, w_attn, w_proj):
    xT = np.ascontiguousarray(
        np.asarray(x, np.float32).reshape(BT, D).T
    ).astype(BF16NP)  # [D, BT]
    in_maps = []
    for c in range(NCORES):
        c0 = c * CW
        wq = w_attn[:, c0 : c0 + CW]
        wk = w_attn[:, D + c0 : D + c0 + CW]
        wv = w_attn[:, 2 * D + c0 : 2 * D + c0 + CW]
        wslice = np.concatenate([wq, wk, wv], axis=1)          # [D, 3*CW]
        wpacked = np.ascontiguousarray(
            np.asarray(wslice, np.float32)
            .reshape(KC, 128, 3 * CW)
            .transpose(1, 0, 2)
        ).reshape(128, KC * 3 * CW).astype(BF16NP)
        wpc = np.ascontiguousarray(
            np.asarray(w_proj[c0 : c0 + CW, :], np.float32)
        ).astype(BF16NP)                                        # [CW, D]
        in_maps.append({"xT": xT, "wqkv": wpacked, "wp": wpc})
    return in_maps


def kernel(x: np.ndarray, w_attn: np.ndarray, w_proj: np.ndarray) -> np.ndarray:
    global _NC_CACHE
    if _NC_CACHE is None:
        _NC_CACHE = build_kernel()
    nc = _NC_CACHE

    in_maps = make_in_maps(x, w_attn, w_proj)
    res = run_bass_kernel_spmd(nc, in_maps, core_ids=list(range(NCORES)))
    acc = np.zeros((BT, D), dtype=np.float32)
    for r in res.results:
        acc += np.asarray(r["out"], dtype=np.float32)
    return acc.reshape(B, T, D)


if __name__ == "__main__":
    inputs = {
        "x": np.random.randn(B, T, D).astype(np.float32),
        "w_attn": (np.random.randn(D, 3 * D) / np.sqrt(D)).astype(np.float32),
        "w_proj": (np.random.randn(D, D) / np.sqrt(D)).astype(np.float32),
    }
    y = kernel(**inputs)
    print(y.shape, y.dtype)


# revision 13
# speedup vs baseline: 1.2446x; 1.0053x over previous
"""Causal self-attention on 8 NeuronCores (TRN2), tensor-parallel over heads.

Reference: y = proj(softmax(causal(Q K^T / sqrt(64))) V) with
B=4, T=2048, D=1024, H=16 heads, head_dim=64.

Sharding: each core owns 2 heads (a 128-column slice of the Q/K/V
projections and the matching 128 rows of w_proj) for all batches. Each
core emits a partial [B*T, D] output; the host sums the 8 partials
(row-parallel matmul unshard) and reshapes to [B, T, D].

All PE operands are bf16 (fp32 PSUM accumulation). V is produced
directly in token-major layout (x chunk stationary), so no PE
transposes are needed. The whole schedule is software-pipelined: the
attention kk-loop for batch b emits S of step i+1 before AV of step i,
and qkv(b+1) / out_proj(b-1) matmul groups are interleaved into the
stream as PE filler so the tensor engine never idles behind the
scalar-engine exp (idle PE triggers HAM re-throttle to half clock).
The softmax divide of each batch's last query chunk is carried into
the next batch's stream so its reciprocal never blocks the gpsimd
queue. Only Exp/Copy activations are used - a single table load.
"""

import sys

for _p in ("/opt/trn_rl_repo",):
    if _p not in sys.path:
        sys.path.insert(0, _p)

import ml_dtypes
import numpy as np

import concourse.bass as bass
import concourse.bacc as bacc
import concourse.mybir as mybir
from concourse import tile
from concourse.bass_utils import run_bass_kernel_spmd

B, T, D, H = 4, 2048, 1024, 16
HD = D // H           # 64 head dim
NCORES = 8
HPC = H // NCORES     # 2 heads per core
CW = HPC * HD         # 128: per-core qkv column slice width
BT = B * T            # 8192 tokens
KC = D // 128         # 8 contraction chunks for the qkv projection
NQ = 512              # query chunk
NG = NQ // 128        # 4 key-tiles per S^T group
F32 = mybir.dt.float32
BF16 = mybir.dt.bfloat16
EXP = mybir.ActivationFunctionType.Exp
BF16NP = np.dtype(ml_dtypes.bfloat16)

VST = HPC * (HD + 1)  # 130: V tile stride (per head: 64 cols + ones col)
NSTEP = sum(NG * (jq + 1) for jq in range(T // NQ))  # 40 attention steps/batch


def build_kernel():
    nc = bacc.Bacc("TRN2", target_bir_lowering=False, debug=False)

    xT = nc.dram_tensor("xT", [D, BT], BF16, kind="ExternalInput")
    # wqkv packed on host as [128, KC, 3*CW]: (kc,:) = rows kc*128..+128 of
    # [w_q_slice | w_k_slice | w_v_slice]
    wqkv = nc.dram_tensor("wqkv", [128, KC * 3 * CW], BF16, kind="ExternalInput")
    wp = nc.dram_tensor("wp", [CW, D], BF16, kind="ExternalInput")
    out = nc.dram_tensor("out", [BT, D], BF16, kind="ExternalOutput")

    with tile.TileContext(nc) as tc:
        _body(tc, xT.ap(), wqkv.ap(), wp.ap(), out.ap())
    nc.compile()
    return nc


def _body(tc, xT, wqkv, wp, out):
    nc = tc.nc
    with (
        tc.tile_pool(name="const", bufs=1) as const,
        tc.tile_pool(name="xin", bufs=2) as xin,
        tc.tile_pool(name="qk", bufs=2) as qkpool,
        tc.tile_pool(name="vb", bufs=2) as vbpool,
        tc.tile_pool(name="pt", bufs=3) as ptpool,
        tc.tile_pool(name="yt", bufs=2) as ytpool,
        tc.tile_pool(name="dn", bufs=2) as dnpool,
        tc.tile_pool(name="os", bufs=2) as ospool,
        tc.tile_pool(name="pst", bufs=2, space="PSUM") as pst,
        tc.tile_pool(name="pav", bufs=1, space="PSUM") as pav,
        tc.tile_pool(name="psm", bufs=2, space="PSUM") as psm,
    ):
        # ---- constants ----
        wq_sb = const.tile([128, KC, 3 * CW], BF16, tag="wqkv")
        nc.sync.dma_start(wq_sb[:], wqkv.rearrange("p (k c) -> p k c", k=KC))
        wp_sb = const.tile([CW, D], BF16, tag="wp")
        nc.sync.dma_start(wp_sb[:], wp[:])
        ones32 = const.tile([128, (T // 128) * HPC], BF16, tag="ones32")
        nc.gpsimd.memset(ones32[:], 1.0)
        # causal mask for the 128-wide diagonal blocks: mask[p, j] = j >= p,
        # stored twice so one DVE multiply covers both heads' strided blocks
        mask2 = const.tile([128, HPC * 128], BF16, tag="mask2")
        nc.gpsimd.memset(mask2[:], 1.0)
        nc.gpsimd.affine_select(
            out=mask2[:],
            in_=mask2[:],
            pattern=[[0, HPC], [1, 128]],
            channel_multiplier=-1,
            base=0,
            compare_op=mybir.AluOpType.is_ge,
            fill=0.0,
        )
        scale = 1.0 / float(np.sqrt(HD))

        xts, qkvs, yts = {}, {}, {}

        def load_chunks(b):
            def go():
                xt = xin.tile([128, KC, T], BF16, tag="xt")
                xts[b] = xt
                tok0 = b * T
                for kc in range(KC):
                    nc.sync.dma_start(
                        xt[:, kc, :],
                        xT[kc * 128 : (kc + 1) * 128, tok0 : tok0 + T],
                    )
            return [go]

        def qkv_chunks(b):
            """Closures, each emitting one PSUM-group of qkv(b) work."""
            def alloc():
                qt = qkpool.tile([128, T], BF16, tag="qt")
                kt = qkpool.tile([128, T], BF16, tag="kt")
                vb = vbpool.tile([128, (T // 128) * VST], BF16, tag="vb")
                qkvs[b] = (qt, kt, vb)

            chunks = [alloc]
            for ch in range(T // NQ):
                # Q^T / K^T m-tiles (weight-stationary, token-moving)
                for m in (0, 1):
                    def qk_go(ch=ch, m=m):
                        qt, kt, vb = qkvs[b]
                        xt = xts[b]
                        sl = slice(ch * NQ, (ch + 1) * NQ)
                        ps = psm.tile([128, NQ], F32, tag="ps")
                        for kc in range(KC):
                            nc.tensor.matmul(
                                ps[:],
                                wq_sb[:, kc, m * CW : (m + 1) * CW],
                                xt[:, kc, sl],
                                start=(kc == 0),
                                stop=(kc == KC - 1),
                            )
                        nc.vector.tensor_copy((qt, kt)[m][:, sl], ps[:])
                    chunks.append(qk_go)
                # V in token-major layout directly: x 128-token blocks
                # stationary, w_v moving; 4 blocks accumulate into one psum
                # tile, one strided scatter into vb per 512-token chunk
                def v_go(ch=ch):
                    qt, kt, vb = qkvs[b]
                    xt = xts[b]
                    pv = psm.tile([128, NQ], F32, tag="ps")
                    for blk in range(NG):
                        t0 = ch * NQ + blk * 128
                        for kc in range(KC):
                            nc.tensor.matmul(
                                pv[:, blk * CW : (blk + 1) * CW],
                                xt[:, kc, t0 : t0 + 128],
                                wq_sb[:, kc, 2 * CW : 3 * CW],
                                start=(kc == 0),
                                stop=(kc == KC - 1),
                            )
                    dstv = bass.AP(
                        vb.tensor,
                        vb[:].offset + ch * NG * VST,
                        [vb[:].ap[0], [VST, NG], [HD + 1, HPC], [1, HD]],
                    )
                    srcv = pv[:].rearrange("p (t h d) -> p t h d", t=NG, h=HPC)
                    nc.vector.tensor_copy(dstv, srcv)
                chunks.append(v_go)

            def ones_go():
                qt, kt, vb = qkvs[b]
                onesv = bass.AP(
                    vb.tensor,
                    vb[:].offset + HD,
                    [vb[:].ap[0], [HD + 1, (T // 128) * HPC]],
                )
                nc.vector.tensor_copy(onesv, ones32[:])
            chunks.append(ones_go)
            return chunks

        def finalize_pieces(yt, jq, ytu):
            # divide O^T rows by the denominator row (broadcast to 64 parts).
            # Emitted as closures dripped one-per-step into the stream so the
            # reciprocal does not convoy PSUM evacuations on the DVE queue.
            q0 = jq * NQ
            dn = dnpool.tile([1, HPC * NQ], F32, tag="dn")
            dnb = dnpool.tile([HD, HPC * NQ], F32, tag="dnb")
            pieces = [
                lambda: nc.vector.reciprocal(dn[:], ytu[HD : HD + 1, :]),
                lambda: nc.gpsimd.partition_broadcast(dnb[:], dn[:]),
            ]
            for h in range(HPC):
                pieces.append(lambda h=h: nc.gpsimd.tensor_mul(
                    yt[h * HD : (h + 1) * HD, q0 : q0 + NQ],
                    ytu[0:HD, h * NQ : (h + 1) * NQ],
                    dnb[:, h * NQ : (h + 1) * NQ],
                ))
            return pieces

        def outproj_chunks(b):
            chunks = []
            for tt in range(T // 128):
                def go(tt=tt):
                    yt = yts[b]
                    tok0 = b * T
                    os_ = ospool.tile([128, D], BF16, tag="os")
                    for nn in range(D // NQ):
                        pp = psm.tile([128, NQ], F32, tag="ps")
                        nc.tensor.matmul(
                            pp[:],
                            yt[:, tt * 128 : (tt + 1) * 128],
                            wp_sb[:, nn * NQ : (nn + 1) * NQ],
                            start=True,
                            stop=True,
                        )
                        nc.scalar.copy(os_[:, nn * NQ : (nn + 1) * NQ], pp[:])
                    nc.sync.dma_start(
                        out[tok0 + tt * 128 : tok0 + (tt + 1) * 128, :], os_[:]
                    )
                chunks.append(go)
            return chunks

        def attention_stream(b, yt, state):
            """Generator: yields after each kk step so the driver can
            interleave filler (qkv of b+1 / out-proj of b-1) into the PE
            stream. h0 lives in SBUF partitions 0-63, h1 in 64-127; the
            S-exp of step i+1 is emitted before the AV of step i."""
            qt, kt, vb = qkvs[b]

            def emit_s(jq, kk):
                q0 = jq * NQ
                i = kk - NG * jq        # >= 0 on the diagonal run
                c0 = max(i, 0) * 128    # first valid q col in this chunk
                w = NQ - c0
                st = pst.tile([128, HPC * NQ], F32, tag="st")
                for h in range(HPC):
                    nc.tensor.matmul(
                        st[:, h * NQ + c0 : (h + 1) * NQ],
                        kt[h * HD : (h + 1) * HD, kk * 128 : (kk + 1) * 128],
                        qt[h * HD : (h + 1) * HD, q0 + c0 : q0 + NQ],
                        start=True,
                        stop=True,
                    )
                ptk = ptpool.tile([128, HPC * NQ], BF16, tag="pt")
                stv = bass.AP(st.tensor, st[:].offset + c0,
                              [st[:].ap[0], [NQ, HPC], [1, w]])
                ptv = bass.AP(ptk.tensor, ptk[:].offset + c0,
                              [ptk[:].ap[0], [NQ, HPC], [1, w]])
                nc.scalar.activation(ptv, stv, EXP, scale=scale)
                if i >= 0:
                    # zero q < kpart inside the 128-wide diagonal block via a
                    # DVE mask multiply - the gpsimd DSP is far too slow at
                    # semaphore waits to sit on the S->exp->mask->AV path
                    tri = bass.AP(ptk.tensor, ptk[:].offset + c0,
                                  [ptk[:].ap[0], [NQ, HPC], [1, 128]])
                    nc.vector.tensor_mul(
                        tri, tri,
                        mask2[:].rearrange("p (h j) -> p h j", h=HPC),
                    )
                return jq, kk, ptk, c0

            def emit_av(jq, kk, ptk, c0):
                nkk = NG * (jq + 1)
                if kk == 0:
                    av0 = pav.tile([128, NQ], F32, tag="av0")
                    av1 = pav.tile([128, NQ], F32, tag="av1")
                    state["avs"] = (av0, av1)
                avs = state["avs"]
                for h in range(HPC):
                    nc.tensor.matmul(
                        avs[h][0 : HD + 1, c0:NQ],
                        vb[:, kk * VST + h * (HD + 1) :
                             kk * VST + (h + 1) * (HD + 1)],
                        ptk[:, h * NQ + c0 : (h + 1) * NQ],
                        start=(kk == 0),
                        stop=(kk == nkk - 1),
                    )
                if kk == nkk - 1:
                    # evacuate PSUM promptly (frees the av slots); rows 0..63
                    # are the unnormalized O^T, row 64 the denominator
                    ytu = dnpool.tile([HD + 1, HPC * NQ], F32, tag="ytu")
                    nc.vector.tensor_copy(
                        ytu[0 : HD + 1, 0:NQ], avs[0][0 : HD + 1, :]
                    )
                    nc.scalar.copy(
                        ytu[0 : HD + 1, NQ : 2 * NQ], avs[1][0 : HD + 1, :]
                    )
                    if jq == T // NQ - 1:
                        state["pending"] = (jq, ytu)
                    else:
                        state["npieces"].extend(finalize_pieces(yt, jq, ytu))

            prev = None
            idx = 0
            for jq in range(T // NQ):
                for kk in range(NG * (jq + 1)):
                    cur = emit_s(jq, kk)
                    if prev is not None:
                        emit_av(*prev)
                    prev = cur
                    yield
            emit_av(*prev)

        # ---- prologue: x(0), x(1) loads and qkv(0) emitted straight ----
        for go in load_chunks(0) + load_chunks(1) + qkv_chunks(0):
            go()

        carry = None
        for b in range(B):
            yt = ytpool.tile([128, T], BF16, tag="yt")
            yts[b] = yt
            fillers = []
            if b + 2 < B:
                fillers += load_chunks(b + 2)
            if b + 1 < B:
                fillers += qkv_chunks(b + 1)
            if b - 1 >= 0:
                fillers += outproj_chunks(b - 1)
            state = {"pending": None, "avs": None, "npieces": []}
            fi, step = 0, 0
            for _ in attention_stream(b, yt, state):
                step += 1
                if step == NG and carry is not None:
                    # previous batch's last softmax divide enters the drip
                    # queue after this batch's first diagonal masks
                    state["npieces"].extend(finalize_pieces(*carry))
                    carry = None
                if state["npieces"]:
                    state["npieces"].pop(0)()
                # proportional pacing keeps late-dependency fillers late
                while fi * NSTEP < len(fillers) * step and fi < len(fillers):
                    fillers[fi]()
                    fi += 1
            for go in state["npieces"]:
                go()
            while fi < len(fillers):
                fillers[fi]()
                fi += 1
            jq3, ytu3 = state["pending"]
            carry = (yt, jq3, ytu3)

        # ---- epilogue ----
        for go in finalize_pieces(*carry):
            go()
        for go in outproj_chunks(B - 1):
            go()


_NC_CACHE = None


def make_in_maps(x, w_attn, w_proj):
    xT = np.ascontiguousarray(
        np.asarray(x, np.float32).reshape(BT, D).T
    ).astype(BF16NP)  # [D, BT]
    in_maps = []
    for c in range(NCORES):
        c0 = c * CW
        wq = w_attn[:, c0 : c0 + CW]
        wk = w_attn[:, D + c0 : D + c0 + CW]
        wv = w_attn[:, 2 * D + c0 : 2 * D + c0 + CW]
        wslice = np.concatenate([wq, wk, wv], axis=1)          # [D, 3*CW]
        wpacked = np.ascontiguousarray(
            np.asarray(wslice, np.float32)
            .reshape(KC, 128, 3 * CW)
            .transpose(1, 0, 2)
        ).reshape(128, KC * 3 * CW).astype(BF16NP)
        wpc = np.ascontiguousarray(
            np.asarray(w_proj[c0 : c0 + CW, :], np.float32)
        ).astype(BF16NP)                                        # [CW, D]
        in_maps.append({"xT": xT, "wqkv": wpacked, "wp": wpc})
    return in_maps


def kernel(x: np.ndarray, w_attn: np.ndarray, w_proj: np.ndarray) -> np.ndarray:
    global _NC_CACHE
    if _NC_CACHE is None:
        _NC_CACHE = build_kernel()
    nc = _NC_CACHE

    in_maps = make_in_maps(x, w_attn, w_proj)
    res = run_bass_kernel_spmd(nc, in_maps, core_ids=list(range(NCORES)))
    acc = np.zeros((BT, D), dtype=np.float32)
    for r in res.results:
        acc += np.asarray(r["out"], dtype=np.float32)
    return acc.reshape(B, T, D)


if __name__ == "__main__":
    inputs = {
        "x": np.random.randn(B, T, D).astype(np.float32),
        "w_attn": (np.random.randn(D, 3 * D) / np.sqrt(D)).astype(np.float32),
        "w_proj": (np.random.randn(D, D) / np.sqrt(D)).astype(np.float32),
    }
    y = kernel(**inputs)
    print(y.shape, y.dtype)


# revision 14
# speedup vs baseline: 1.2539x; 1.0075x over previous
"""Causal self-attention on 8 NeuronCores (TRN2), tensor-parallel over heads.

Reference: y = proj(softmax(causal(Q K^T / sqrt(64))) V) with
B=4, T=2048, D=1024, H=16 heads, head_dim=64.

Sharding: each core owns 2 heads (a 128-column slice of the Q/K/V
projections and the matching 128 rows of w_proj) for all batches. Each
core emits a partial [B*T, D] output; the host sums the 8 partials
(row-parallel matmul unshard) and reshapes to [B, T, D].

All PE operands are bf16 (fp32 PSUM accumulation). V is produced
directly in token-major layout (x chunk stationary), so no PE
transposes are needed. The whole schedule is software-pipelined: the
attention kk-loop for batch b emits S of step i+1 before AV of step i,
and qkv(b+1) / out_proj(b-1) matmul groups are interleaved into the
stream as PE filler so the tensor engine never idles behind the
scalar-engine exp (idle PE triggers HAM re-throttle to half clock).
The softmax divide of each batch's last query chunk is carried into
the next batch's stream so its reciprocal never blocks the gpsimd
queue. Only Exp/Copy activations are used - a single table load.
"""

import sys

for _p in ("/opt/trn_rl_repo",):
    if _p not in sys.path:
        sys.path.insert(0, _p)

import ml_dtypes
import numpy as np

import concourse.bass as bass
import concourse.bacc as bacc
import concourse.mybir as mybir
from concourse import tile
from concourse.bass_utils import run_bass_kernel_spmd

B, T, D, H = 4, 2048, 1024, 16
HD = D // H           # 64 head dim
NCORES = 8
HPC = H // NCORES     # 2 heads per core
CW = HPC * HD         # 128: per-core qkv column slice width
BT = B * T            # 8192 tokens
KC = D // 128         # 8 contraction chunks for the qkv projection
NQ = 512              # query chunk
NG = NQ // 128        # 4 key-tiles per S^T group
F32 = mybir.dt.float32
BF16 = mybir.dt.bfloat16
EXP = mybir.ActivationFunctionType.Exp
BF16NP = np.dtype(ml_dtypes.bfloat16)

VST = HPC * (HD + 1)  # 130: V tile stride (per head: 64 cols + ones col)
NSTEP = sum(NG * (jq + 1) for jq in range(T // NQ))  # 40 attention steps/batch


def build_kernel():
    nc = bacc.Bacc("TRN2", target_bir_lowering=False, debug=False)

    xT = nc.dram_tensor("xT", [D, BT], BF16, kind="ExternalInput")
    # wqkv packed on host as [128, KC, 3*CW]: (kc,:) = rows kc*128..+128 of
    # [w_q_slice | w_k_slice | w_v_slice]
    wqkv = nc.dram_tensor("wqkv", [128, KC * 3 * CW], BF16, kind="ExternalInput")
    wp = nc.dram_tensor("wp", [CW, D], BF16, kind="ExternalInput")
    out = nc.dram_tensor("out", [BT, D], BF16, kind="ExternalOutput")

    with tile.TileContext(nc) as tc:
        _body(tc, xT.ap(), wqkv.ap(), wp.ap(), out.ap())
    nc.compile()
    return nc


def _body(tc, xT, wqkv, wp, out):
    nc = tc.nc
    with (
        tc.tile_pool(name="const", bufs=1) as const,
        tc.tile_pool(name="xin", bufs=2) as xin,
        tc.tile_pool(name="qk", bufs=2) as qkpool,
        tc.tile_pool(name="vb", bufs=2) as vbpool,
        tc.tile_pool(name="pt", bufs=3) as ptpool,
        tc.tile_pool(name="yt", bufs=2) as ytpool,
        tc.tile_pool(name="dn", bufs=2) as dnpool,
        tc.tile_pool(name="os", bufs=2) as ospool,
        tc.tile_pool(name="pst", bufs=2, space="PSUM") as pst,
        tc.tile_pool(name="pav", bufs=1, space="PSUM") as pav,
        tc.tile_pool(name="psm", bufs=2, space="PSUM") as psm,
    ):
        # ---- constants ----
        wq_sb = const.tile([128, KC, 3 * CW], BF16, tag="wqkv")
        nc.sync.dma_start(wq_sb[:], wqkv.rearrange("p (k c) -> p k c", k=KC))
        wp_sb = const.tile([CW, D], BF16, tag="wp")
        nc.sync.dma_start(wp_sb[:], wp[:])
        ones32 = const.tile([128, (T // 128) * HPC], BF16, tag="ones32")
        nc.gpsimd.memset(ones32[:], 1.0)
        # causal mask for the 128-wide diagonal blocks: mask[p, j] = j >= p,
        # stored twice so one DVE multiply covers both heads' strided blocks
        mask2 = const.tile([128, HPC * 128], BF16, tag="mask2")
        nc.gpsimd.memset(mask2[:], 1.0)
        nc.gpsimd.affine_select(
            out=mask2[:],
            in_=mask2[:],
            pattern=[[0, HPC], [1, 128]],
            channel_multiplier=-1,
            base=0,
            compare_op=mybir.AluOpType.is_ge,
            fill=0.0,
        )
        scale = 1.0 / float(np.sqrt(HD))

        warm = psm.tile([128, HPC * 128], F32, tag="ps", name="warm")
        for _ in range(16):
            nc.tensor.matmul(
                warm[:], mask2[:, 0:128], mask2[:], start=True, stop=True
            )

        xts, qkvs, yts = {}, {}, {}

        def load_chunks(b):
            def go():
                xt = xin.tile([128, KC, T], BF16, tag="xt")
                xts[b] = xt
                tok0 = b * T
                for kc in range(KC):
                    nc.sync.dma_start(
                        xt[:, kc, :],
                        xT[kc * 128 : (kc + 1) * 128, tok0 : tok0 + T],
                    )
            return [go]

        def qkv_chunks(b):
            """Closures, each emitting one PSUM-group of qkv(b) work."""
            def alloc():
                qt = qkpool.tile([128, T], BF16, tag="qt")
                kt = qkpool.tile([128, T], BF16, tag="kt")
                vb = vbpool.tile([128, (T // 128) * VST], BF16, tag="vb")
                qkvs[b] = (qt, kt, vb)

            chunks = [alloc]
            for ch in range(T // NQ):
                # Q^T / K^T m-tiles (weight-stationary, token-moving)
                for m in (0, 1):
                    def qk_go(ch=ch, m=m):
                        qt, kt, vb = qkvs[b]
                        xt = xts[b]
                        sl = slice(ch * NQ, (ch + 1) * NQ)
                        ps = psm.tile([128, NQ], F32, tag="ps")
                        for kc in range(KC):
                            nc.tensor.matmul(
                                ps[:],
                                wq_sb[:, kc, m * CW : (m + 1) * CW],
                                xt[:, kc, sl],
                                start=(kc == 0),
                                stop=(kc == KC - 1),
                            )
                        nc.vector.tensor_copy((qt, kt)[m][:, sl], ps[:])
                    chunks.append(qk_go)
                # V in token-major layout directly: x 128-token blocks
                # stationary, w_v moving; 4 blocks accumulate into one psum
                # tile, one strided scatter into vb per 512-token chunk
                def v_go(ch=ch):
                    qt, kt, vb = qkvs[b]
                    xt = xts[b]
                    pv = psm.tile([128, NQ], F32, tag="ps")
                    for blk in range(NG):
                        t0 = ch * NQ + blk * 128
                        for kc in range(KC):
                            nc.tensor.matmul(
                                pv[:, blk * CW : (blk + 1) * CW],
                                xt[:, kc, t0 : t0 + 128],
                                wq_sb[:, kc, 2 * CW : 3 * CW],
                                start=(kc == 0),
                                stop=(kc == KC - 1),
                            )
                    dstv = bass.AP(
                        vb.tensor,
                        vb[:].offset + ch * NG * VST,
                        [vb[:].ap[0], [VST, NG], [HD + 1, HPC], [1, HD]],
                    )
                    srcv = pv[:].rearrange("p (t h d) -> p t h d", t=NG, h=HPC)
                    nc.vector.tensor_copy(dstv, srcv)
                chunks.append(v_go)

            def ones_go():
                qt, kt, vb = qkvs[b]
                onesv = bass.AP(
                    vb.tensor,
                    vb[:].offset + HD,
                    [vb[:].ap[0], [HD + 1, (T // 128) * HPC]],
                )
                nc.vector.tensor_copy(onesv, ones32[:])
            chunks.append(ones_go)
            return chunks

        def finalize_pieces(yt, jq, ytu):
            # divide O^T rows by the denominator row (broadcast to 64 parts).
            # Emitted as closures dripped one-per-step into the stream so the
            # reciprocal does not convoy PSUM evacuations on the DVE queue.
            q0 = jq * NQ
            dn = dnpool.tile([1, HPC * NQ], F32, tag="dn")
            dnb = dnpool.tile([HD, HPC * NQ], F32, tag="dnb")
            pieces = [
                lambda: nc.vector.reciprocal(dn[:], ytu[HD : HD + 1, :]),
                lambda: nc.gpsimd.partition_broadcast(dnb[:], dn[:]),
            ]
            for h in range(HPC):
                pieces.append(lambda h=h: nc.gpsimd.tensor_mul(
                    yt[h * HD : (h + 1) * HD, q0 : q0 + NQ],
                    ytu[0:HD, h * NQ : (h + 1) * NQ],
                    dnb[:, h * NQ : (h + 1) * NQ],
                ))
            return pieces

        def outproj_chunks(b):
            chunks = []
            for tt in range(T // 128):
                def go(tt=tt):
                    yt = yts[b]
                    tok0 = b * T
                    os_ = ospool.tile([128, D], BF16, tag="os")
                    for nn in range(D // NQ):
                        pp = psm.tile([128, NQ], F32, tag="ps")
                        nc.tensor.matmul(
                            pp[:],
                            yt[:, tt * 128 : (tt + 1) * 128],
                            wp_sb[:, nn * NQ : (nn + 1) * NQ],
                            start=True,
                            stop=True,
                        )
                        nc.scalar.copy(os_[:, nn * NQ : (nn + 1) * NQ], pp[:])
                    nc.sync.dma_start(
                        out[tok0 + tt * 128 : tok0 + (tt + 1) * 128, :], os_[:]
                    )
                chunks.append(go)
            return chunks

        def attention_stream(b, yt, state):
            """Generator: yields after each kk step so the driver can
            interleave filler (qkv of b+1 / out-proj of b-1) into the PE
            stream. h0 lives in SBUF partitions 0-63, h1 in 64-127; the
            S-exp of step i+1 is emitted before the AV of step i."""
            qt, kt, vb = qkvs[b]

            def emit_s(jq, kk):
                q0 = jq * NQ
                i = kk - NG * jq        # >= 0 on the diagonal run
                c0 = max(i, 0) * 128    # first valid q col in this chunk
                w = NQ - c0
                st = pst.tile([128, HPC * NQ], F32, tag="st")
                for h in range(HPC):
                    nc.tensor.matmul(
                        st[:, h * NQ + c0 : (h + 1) * NQ],
                        kt[h * HD : (h + 1) * HD, kk * 128 : (kk + 1) * 128],
                        qt[h * HD : (h + 1) * HD, q0 + c0 : q0 + NQ],
                        start=True,
                        stop=True,
                    )
                ptk = ptpool.tile([128, HPC * NQ], BF16, tag="pt")
                stv = bass.AP(st.tensor, st[:].offset + c0,
                              [st[:].ap[0], [NQ, HPC], [1, w]])
                ptv = bass.AP(ptk.tensor, ptk[:].offset + c0,
                              [ptk[:].ap[0], [NQ, HPC], [1, w]])
                nc.scalar.activation(ptv, stv, EXP, scale=scale)
                if i >= 0:
                    # zero q < kpart inside the 128-wide diagonal block via a
                    # DVE mask multiply - the gpsimd DSP is far too slow at
                    # semaphore waits to sit on the S->exp->mask->AV path
                    tri = bass.AP(ptk.tensor, ptk[:].offset + c0,
                                  [ptk[:].ap[0], [NQ, HPC], [1, 128]])
                    nc.vector.tensor_mul(
                        tri, tri,
                        mask2[:].rearrange("p (h j) -> p h j", h=HPC),
                    )
                return jq, kk, ptk, c0

            def emit_av(jq, kk, ptk, c0):
                nkk = NG * (jq + 1)
                if kk == 0:
                    av0 = pav.tile([128, NQ], F32, tag="av0")
                    av1 = pav.tile([128, NQ], F32, tag="av1")
                    state["avs"] = (av0, av1)
                avs = state["avs"]
                for h in range(HPC):
                    nc.tensor.matmul(
                        avs[h][0 : HD + 1, c0:NQ],
                        vb[:, kk * VST + h * (HD + 1) :
                             kk * VST + (h + 1) * (HD + 1)],
                        ptk[:, h * NQ + c0 : (h + 1) * NQ],
                        start=(kk == 0),
                        stop=(kk == nkk - 1),
                    )
                if kk == nkk - 1:
                    # evacuate PSUM promptly (frees the av slots); rows 0..63
                    # are the unnormalized O^T, row 64 the denominator
                    ytu = dnpool.tile([HD + 1, HPC * NQ], F32, tag="ytu")
                    nc.scalar.copy(
                        ytu[0 : HD + 1, 0:NQ], avs[0][0 : HD + 1, :]
                    )
                    nc.scalar.copy(
                        ytu[0 : HD + 1, NQ : 2 * NQ], avs[1][0 : HD + 1, :]
                    )
                    if jq == T // NQ - 1:
                        state["pending"] = (jq, ytu)
                    else:
                        state["npieces"].extend(finalize_pieces(yt, jq, ytu))

            prev = None
            idx = 0
            for jq in range(T // NQ):
                for kk in range(NG * (jq + 1)):
                    cur = emit_s(jq, kk)
                    if prev is not None:
                        emit_av(*prev)
                    prev = cur
                    yield
            emit_av(*prev)

        # ---- prologue: x(0), x(1) loads and qkv(0) emitted straight ----
        for go in load_chunks(0) + load_chunks(1) + qkv_chunks(0):
            go()

        carry = None
        for b in range(B):
            yt = ytpool.tile([128, T], BF16, tag="yt")
            yts[b] = yt
            fillers = []
            if b + 2 < B:
                fillers += load_chunks(b + 2)
            if b + 1 < B:
                fillers += qkv_chunks(b + 1)
            if b - 1 >= 0:
                fillers += outproj_chunks(b - 1)
            state = {"pending": None, "avs": None, "npieces": []}
            fi, step = 0, 0
            for _ in attention_stream(b, yt, state):
                step += 1
                if step == NG and carry is not None:
                    # previous batch's last softmax divide enters the drip
                    # queue after this batch's first diagonal masks
                    state["npieces"].extend(finalize_pieces(*carry))
                    carry = None
                if state["npieces"]:
                    state["npieces"].pop(0)()
                # proportional pacing keeps late-dependency fillers late
                while fi * NSTEP < len(fillers) * step and fi < len(fillers):
                    fillers[fi]()
                    fi += 1
            for go in state["npieces"]:
                go()
            while fi < len(fillers):
                fillers[fi]()
                fi += 1
            jq3, ytu3 = state["pending"]
            carry = (yt, jq3, ytu3)

        # ---- epilogue ----
        for go in finalize_pieces(*carry):
            go()
        for go in outproj_chunks(B - 1):
            go()


_NC_CACHE = None


def make_in_maps(x, w_attn, w_proj):
    xT = np.ascontiguousarray(
        np.asarray(x, np.float32).reshape(BT, D).T
    ).astype(BF16NP)  # [D, BT]
    in_maps = []
    for c in range(NCORES):
        c0 = c * CW
        wq = w_attn[:, c0 : c0 + CW]
        wk = w_attn[:, D + c0 : D + c0 + CW]
        wv = w_attn[:, 2 * D + c0 : 2 * D + c0 + CW]
        wslice = np.concatenate([wq, wk, wv], axis=1)          # [D, 3*CW]
        wpacked = np.ascontiguousarray(
            np.asarray(wslice, np.float32)
            .reshape(KC, 128, 3 * CW)
            .transpose(1, 0, 2)
        ).reshape(128, KC * 3 * CW).astype(BF16NP)
        wpc = np.ascontiguousarray(
            np.asarray(w_proj[c0 : c0 + CW, :], np.float32)
        ).astype(BF16NP)                                        # [CW, D]
        in_maps.append({"xT": xT, "wqkv": wpacked, "wp": wpc})
    return in_maps


def kernel(x: np.ndarray, w_attn: np.ndarray, w_proj: np.ndarray) -> np.ndarray:
    global _NC_CACHE
    if _NC_CACHE is None:
        _NC_CACHE = build_kernel()
    nc = _NC_CACHE

    in_maps = make_in_maps(x, w_attn, w_proj)
    res = run_bass_kernel_spmd(nc, in_maps, core_ids=list(range(NCORES)))
    acc = np.zeros((BT, D), dtype=np.float32)
    for r in res.results:
        acc += np.asarray(r["out"], dtype=np.float32)
    return acc.reshape(B, T, D)


if __name__ == "__main__":
    inputs = {
        "x": np.random.randn(B, T, D).astype(np.float32),
        "w_attn": (np.random.randn(D, 3 * D) / np.sqrt(D)).astype(np.float32),
        "w_proj": (np.random.randn(D, D) / np.sqrt(D)).astype(np.float32),
    }
    y = kernel(**inputs)
    print(y.shape, y.dtype)
